# revision 1
# baseline (speedup 1.0000x reference)
"""Trainium2 Bass kernel for nn_ConceptIntergation (histogram_binning).

Reference computation:
    counts[b,s,n] = sum_k one_hot(concepts[b,s,k], 129)[..., n]  (n < 128; 128 = padding)
    out[b,s,n,d]  = counts[b,s,n] * emb_table[n,d]

Strategy (data-parallel over batch, 8 cores):
  - Each core handles B_LOC=8 batches -> 1600 (b,s) rows, output shard
    [1600, 128*64] f32 (~52 MB). The kernel is HBM-write bound; the whole
    design keeps the store stream saturated from ~10us to the end.
  - Rows are processed in 128-row blocks (s on partitions). Histogram via
    iota-compare on DVE (tensor_scalar is_equal + scalar_tensor_tensor
    accumulate), then broadcast tensor_tensor multiplies produce
    [128, 2048] chunks = counts[:,n] * emb[n,d]; each chunk is a 1 MB DMA
    store (contiguous 8 KB per partition).
  - The embedding table is loaded once as a single 32 KB row and
    replicated across partitions on-device by GpSimd partition_broadcast,
    chunk by chunk, so the first multiplies start early and no 4 MB
    replica load competes with the store stream.
"""

import numpy as np

import concourse.bass as bass
import concourse.mybir as mybir
from concourse import bacc
from concourse.tile import TileContext
from concourse.bass_utils import run_bass_kernel_spmd

B, S, K = 64, 200, 4
N, D = 128, 64
ND = N * D                      # 8192
NCORES = 8
B_LOC = B // NCORES             # 8
ROWS = B_LOC * S                # 1600 (b,s) rows per core
P = 128
NBLK = (ROWS + P - 1) // P      # 13 (12 full + 1 of 64 rows)

CH = 4                          # emb-replica/mul/store chunks per block
CW = ND // CH                   # 2048 cols per chunk (= 16 n-rows), 1 MB stores
NCH = N // CH                   # 16 n-rows per chunk

_NC_CACHE = {}


def _build_nc():
    nc = bacc.Bacc()
    idx = nc.declare_dram_parameter("idx", [P, NBLK * K], mybir.dt.float32, isOutput=False)
    embrep = nc.declare_dram_parameter("embrep", [P, ND], mybir.dt.float32, isOutput=False)
    iota = nc.declare_dram_parameter("iota", [P, N], mybir.dt.float32, isOutput=False)
    out = nc.declare_dram_parameter("out", [ROWS, ND], mybir.dt.float32, isOutput=True)

    with TileContext(nc) as tc:
        with (
            tc.tile_pool(name="const", bufs=1) as cpool,
            tc.tile_pool(name="counts", bufs=NBLK) as hpool,
            tc.tile_pool(name="work", bufs=12) as wpool,
        ):
            # small inputs first so the first histogram can start immediately
            iota_sb = cpool.tile([P, N], mybir.dt.float32)
            nc.sync.dma_start(out=iota_sb, in_=iota[:, :])
            idx_sb = cpool.tile([P, NBLK * K], mybir.dt.float32)
            nc.sync.dma_start(out=idx_sb, in_=idx[:, :])
            # embedding replica loaded in chunks; chunk 0 lands first and
            # unblocks the first multiplies while the rest stream in during
            # the ramp, before the store stream saturates HBM.
            emb_sb = cpool.tile([P, ND], mybir.dt.float32)
            for c in range(CH):
                nc.sync.dma_start(
                    out=emb_sb[:, c * CW : (c + 1) * CW],
                    in_=embrep[:, c * CW : (c + 1) * CW],
                )

            def emit_hist(j, counts, pj):
                nc.vector.tensor_scalar(
                    out=counts[:pj],
                    in0=iota_sb[:pj],
                    scalar1=idx_sb[:pj, j * K : j * K + 1],
                    scalar2=None,
                    op0=mybir.AluOpType.is_equal,
                )
                for k in range(1, K):
                    nc.vector.scalar_tensor_tensor(
                        out=counts[:pj],
                        in0=iota_sb[:pj],
                        scalar=idx_sb[:pj, j * K + k : j * K + k + 1],
                        in1=counts[:pj],
                        op0=mybir.AluOpType.is_equal,
                        op1=mybir.AluOpType.add,
                    )

            def emit_mul(j, c, counts, pj):
                ot = wpool.tile([P, CW], mybir.dt.float32, tag="ot")
                nc.vector.tensor_tensor(
                    out=ot[:pj].rearrange("p (n d) -> p n d", d=D),
                    in0=counts[:pj, c * NCH : (c + 1) * NCH, None].broadcast_to(
                        [pj, NCH, D]
                    ),
                    in1=emb_sb[:pj, c * CW : (c + 1) * CW].rearrange(
                        "p (n d) -> p n d", d=D
                    ),
                    op=mybir.AluOpType.mult,
                )
                nc.sync.dma_start(
                    out=out[j * P : j * P + pj, c * CW : (c + 1) * CW],
                    in_=ot[:pj],
                )

            # chunk-major: the c=0 stripe (gated only on the small HBM
            # replica chunk) runs first, hiding the on-device broadcast
            # latency of chunks 1..3 behind ~40us of DVE work. Histograms
            # are interleaved into the first stripe so the first store
            # issues as early as possible.
            counts_tiles = [None] * NBLK
            for j in range(NBLK):
                pj = min(P, ROWS - j * P)
                counts = hpool.tile([P, N], mybir.dt.float32, tag="counts")
                counts_tiles[j] = counts
                emit_hist(j, counts, pj)
                emit_mul(j, 0, counts, pj)
            for c in range(1, CH):
                for j in range(NBLK):
                    pj = min(P, ROWS - j * P)
                    emit_mul(j, c, counts_tiles[j], pj)

    nc.finalize()
    return nc


def _get_nc():
    if "nc" not in _NC_CACHE:
        _NC_CACHE["nc"] = _build_nc()
    return _NC_CACHE["nc"]


def _prepare_in_maps(concepts, emb_table):
    concepts = np.asarray(concepts)
    emb = np.ascontiguousarray(np.asarray(emb_table, dtype=np.float32).reshape(1, ND))

    # per-core index shards, padded to NBLK*P rows, laid out [P, NBLK*K]
    conc = concepts.reshape(NCORES, ROWS, K).astype(np.float32)
    idx_pad = np.full((NCORES, NBLK * P, K), float(N), dtype=np.float32)
    idx_pad[:, :ROWS] = conc
    # [core, NBLK, P, K] -> [core, P, NBLK*K]
    idx_dev = np.ascontiguousarray(
        idx_pad.reshape(NCORES, NBLK, P, K).transpose(0, 2, 1, 3).reshape(NCORES, P, NBLK * K)
    )

    iota = np.ascontiguousarray(
        np.broadcast_to(np.arange(N, dtype=np.float32), (P, N))
    )
    embrep = np.ascontiguousarray(np.broadcast_to(emb, (P, ND)))
    return [
        {"idx": idx_dev[i], "embrep": embrep, "iota": iota}
        for i in range(NCORES)
    ]


def _run(concepts, emb_table, **spmd_kwargs):
    nc = _get_nc()
    in_maps = _prepare_in_maps(concepts, emb_table)
    res = run_bass_kernel_spmd(nc, in_maps, core_ids=list(range(NCORES)), **spmd_kwargs)
    out = np.concatenate(
        [res.results[i]["out"].reshape(B_LOC, S, N, D) for i in range(NCORES)],
        axis=0,
    )
    return out, res


def kernel(concepts, emb_table):
    out, _ = _run(concepts, emb_table)
    return out

